# revision 2
# baseline (speedup 1.0000x reference)
"""Trainium2 Bass kernel for nn_CAM (DANet channel-attention module).

Per batch element b (one per NeuronCore, 8 cores data-parallel over B=8):
    xf = x[b].reshape(C, H*W)                       # [512, 4096]
    E = xf @ xf.T                                   # [512, 512] (symmetric)
    att = softmax(max_j(E) - E, axis=-1)            # inverted softmax
    out = gamma * (att @ xf) + x[b]

Kernel math (identical in exact arithmetic to the reference):
    c[i]    = min_j E[i, j]         (= column min by symmetry)
    V[i, j] = exp(c[i] - E[i, j])   (exponent <= 0; att numerator^T)
    S[i]    = sum_j V[i, j]
    W       = V^T                   (= att numerator in [j, i] layout)
    out[i]  = gamma * (1/S[i]) * sum_j W[j, i] * xf[j, :] + x[b][i, :]

Device pipeline (f32r matmuls; bf16/fp8 E is numerically unacceptable —
the inverted softmax is a near-one-hot argmin over energies with tie
gaps ~1.0 vs E magnitude ~4096, so E needs ~f32 precision):
  - xf natural [c_part, n_free]  : [128, 4, 4096] f32 (residual source)
  - Xf = f32r-rounded copy       : transposes + mm2 rhs (the BIR
                                   verifier requires explicit rounding)
  - xf^T [n_part, c_free]        : rolling [128, 32, 512] f32r via PE
                                   transposes, fused with chunked loads
  - E [j_part, i_free]           : 4 PSUM banks, f32r matmuls over 32
                                   k-tiles; only the upper block-triangle
                                   (free dims >= 256 keeps f32r at
                                   1 cyc/row); lower blocks transposed in
  - V = exp(rowmin - E)          : scalar engine, per-partition bias;
                                   accum_out produces S in the same pass
                                   (no ones-matmuls, no DRAM roundtrip
                                   for a free-axis softmax shift)
  - W = V^T                      : 16 PE transposes, per i-block, so
                                   mm2's first weights are ready ~1.5us
                                   after the last k-tile
  - out                          : mm2 f32r + scalar_tensor_tensor
                                   (po * (gamma/S) + x); with gamma=0 the
                                   output is bit-exactly x

Host path: a persistent jitted shard_map runner (module-cached, built
once), device-resident zero output operands (no 64MB upload per call),
and no host-side stack/astype copies (input reshape and output reshape
are views). Falls back to concourse.bass_utils.run_bass_kernel_spmd on
any failure.

reps > 1 chains the computation through a DRAM scratch tile inside one
NEFF (used by test.py to measure steady-state per-iteration device
time).
"""

import numpy as np

import concourse.bass as bass
import concourse.mybir as mybir
import concourse.tile as tile
from concourse import bacc
from concourse.masks import make_identity

P = 128          # partitions
C = 512          # channels
HW = 4096        # spatial (64*64)
CB = C // P      # 4 channel blocks
KB = HW // P     # 32 spatial blocks
NW = 512         # matmul free-dim chunk
NCH = HW // NW   # 8 n-chunks

F32 = mybir.dt.float32
F32R = mybir.dt.float32r
EXP = mybir.ActivationFunctionType.Exp
ALU = mybir.AluOpType
AX = mybir.AxisListType


def build_nc(reps: int = 1):
    nc = bacc.Bacc("TRN2", target_bir_lowering=False)
    x = nc.dram_tensor("x", [C, HW], F32, kind="ExternalInput")
    g = nc.dram_tensor("gamma", [1], F32, kind="ExternalInput")
    y = nc.dram_tensor("y", [C, HW], F32, kind="ExternalOutput")

    with tile.TileContext(nc) as tc:
        with (
            tc.tile_pool(name="xin", bufs=1) as xin_pool,
            tc.tile_pool(name="xrp", bufs=1) as xr_pool,
            tc.tile_pool(name="xtr", bufs=6) as xtr_pool,
            tc.tile_pool(name="w", bufs=1) as w_pool,
            tc.tile_pool(name="small", bufs=1) as small,
            tc.tile_pool(name="outp", bufs=2) as outp,
            tc.tile_pool(name="dram", bufs=1, space="DRAM") as dramp,
            tc.tile_pool(name="pxt", bufs=2, space="PSUM") as pxt_pool,
            tc.tile_pool(name="acc", bufs=4, space="PSUM") as acc_pool,
        ):
            # constants (hoisted out of the rep loop)
            ident_f = small.tile([P, P], F32)
            make_identity(nc, ident_f)
            ident_r = small.tile([P, P], F32R)
            nc.scalar.copy(out=ident_r, in_=ident_f)
            gamma_bc = small.tile([P, 1], F32)
            nc.gpsimd.dma_start(out=gamma_bc, in_=g[:].partition_broadcast(P))

            xr = x.rearrange("(t p) n -> p t n", p=P)
            yr = y.rearrange("(t p) n -> p t n", p=P)

            if reps > 1:
                ybuf = dramp.tile([C, HW], F32, tag="ybuf")
                ybr = ybuf.rearrange("(t p) n -> p t n", p=P)

            for _rep in range(reps):
                in_r = xr if _rep == 0 else ybr
                out_r = yr if _rep == reps - 1 else ybr
                X = xin_pool.tile([P, CB, HW], F32, tag="x")
                Xf = xr_pool.tile([P, CB, HW], F32R, tag="xr")
                W = w_pool.tile([P, CB, C], F32R, tag="w")
                V = w_pool.tile([P, CB, C], F32R, tag="v")
                rowmin = small.tile([P, CB], F32, tag="rowmin")
                s_acc = small.tile([P, CB], F32, tag="sacc")
                invsg = small.tile([P, CB], F32, tag="invsg")

                # E accumulator banks (held across the fused load/T/mm1 loop)
                pe_tiles = [acc_pool.tile([P, C], F32, tag="acc", name=f"pe_{_jb}")
                            for _jb in range(CB)]
                # by symmetry only the upper block-triangle of E is computed
                # by matmuls; rhs column start per j-block (block (3,2) is
                # recomputed directly so every matmul keeps free dim >= 256)
                RS = (0, P, 2 * P, 2 * P)

                # ---- fused: load chunk -> round to f32r -> transposes -> mm1
                for ch in range(NCH):
                    subs = (
                        (slice(0, NW // 2), slice(NW // 2, NW))
                        if ch == 0 else (slice(0, NW),)
                    )
                    for sub in subs:
                        lo = ch * NW + sub.start
                        hi = ch * NW + sub.stop
                        nc.sync.dma_start(
                            out=X[:, :, lo:hi], in_=in_r[:, :, lo:hi]
                        )
                        for q in range(CB):
                            nc.vector.tensor_copy(
                                out=Xf[:, q, lo:hi], in_=X[:, q, lo:hi]
                            )
                    for kk2 in range(NW // (2 * P)):
                        k0 = ch * (NW // P) + 2 * kk2
                        pxt = pxt_pool.tile([P, 2, C], F32R, tag="pxt")
                        for dk in range(2):
                            for t in range(CB):
                                nc.tensor.transpose(
                                    pxt[:, dk, t * P:(t + 1) * P],
                                    Xf[:, t, (k0 + dk) * P:(k0 + dk + 1) * P],
                                    ident_r,
                                )
                        xt2 = xtr_pool.tile([P, 2, C], F32R, tag="xtk")
                        if kk2 % 2 == 0:
                            nc.vector.tensor_copy(out=xt2, in_=pxt.bitcast(F32))
                        else:
                            nc.scalar.copy(out=xt2, in_=pxt.bitcast(F32))
                        for dk in range(2):
                            k = k0 + dk
                            for jb in range(CB):
                                nc.tensor.matmul(
                                    pe_tiles[jb][:, RS[jb]:],
                                    lhsT=xt2[:, dk, jb * P:(jb + 1) * P],
                                    rhs=xt2[:, dk, RS[jb]:],
                                    start=(k == 0),
                                    stop=(k == KB - 1),
                                )

                # ---- W chain: rowmin -> V = exp(rowmin - E) (+ S accum)
                # -> W = V^T blocks.  Block 0 needs no reconstruction, so its
                # rowmin/exp are emitted first and mm2's ib=0 weights are
                # ready shortly after the last k-tile (no DRAM roundtrip).
                def w_chain(ib):
                    nc.vector.tensor_reduce(
                        out=rowmin[:, ib:ib + 1], in_=pe_tiles[ib],
                        axis=AX.X, op=ALU.min,
                    )
                    nc.scalar.activation(
                        out=V[:, ib, :], in_=pe_tiles[ib], func=EXP,
                        bias=rowmin[:, ib:ib + 1], scale=-1.0,
                        accum_out=s_acc[:, ib:ib + 1],
                    )

                def w_transpose(ib):
                    # lands on pe_tiles[ib]'s PSUM bank (acc pool rotation);
                    # every reader of that bank is emitted above.
                    wtp = acc_pool.tile([P, CB, P], F32R, tag="acc")
                    for jb in range(CB):
                        nc.tensor.transpose(
                            wtp[:, jb, :], V[:, ib, jb * P:(jb + 1) * P], ident_r
                        )
                    isl = slice(ib * P, (ib + 1) * P)
                    if ib % 2 == 0:
                        nc.vector.tensor_copy(
                            out=W[:, :, isl], in_=wtp.bitcast(F32)
                        )
                    else:
                        nc.scalar.copy(out=W[:, :, isl], in_=wtp.bitcast(F32))

                w_chain(0)
                # reconstruct the lower block-triangle: E[i,j] = E[j,i]^T
                blk = small.tile([P, 5, P], F32, tag="blk")
                for n5, (bi, bj) in enumerate(((1, 0), (2, 0), (2, 1), (3, 0), (3, 1))):
                    nc.vector.tensor_copy(
                        out=blk[:, n5, :], in_=pe_tiles[bj][:, bi * P:(bi + 1) * P]
                    )
                    nc.tensor.transpose(
                        pe_tiles[bi][:, bj * P:(bj + 1) * P], blk[:, n5, :], ident_f
                    )
                for ib in range(1, CB):
                    w_chain(ib)
                for ib in range(CB):
                    w_transpose(ib)

                # ---- 1/S * gamma (per-partition, i-indexed)
                nc.vector.reciprocal(out=invsg, in_=s_acc)
                for ib in range(CB):
                    nc.vector.tensor_tensor(
                        out=invsg[:, ib:ib + 1], in0=invsg[:, ib:ib + 1],
                        in1=gamma_bc, op=ALU.mult,
                    )

                # ---- phase 2: out = gamma * (1/S) * (W^T @ xf) + x
                for ib in range(CB):
                    isl = slice(ib * P, (ib + 1) * P)
                    out_sb = outp.tile([P, HW], F32, tag="osb")
                    for chn in range(NCH):
                        nsl = slice(chn * NW, (chn + 1) * NW)
                        po_t = acc_pool.tile([P, NW], F32, tag="acc")
                        for jb in range(CB):
                            nc.tensor.matmul(
                                po_t,
                                lhsT=W[:, jb, isl],
                                rhs=Xf[:, jb, nsl],
                                start=(jb == 0),
                                stop=(jb == CB - 1),
                            )
                        nc.vector.scalar_tensor_tensor(
                            out=out_sb[:, nsl],
                            in0=po_t,
                            scalar=invsg[:, ib:ib + 1],
                            in1=X[:, ib, nsl],
                            op0=ALU.mult,
                            op1=ALU.add,
                        )
                        nc.scalar.dma_start(
                            out=out_r[:, ib, nsl], in_=out_sb[:, nsl]
                        )

    nc.compile()
    return nc


_NC_CACHE = None


def _get_nc():
    global _NC_CACHE
    if _NC_CACHE is None:
        _NC_CACHE = build_nc()
    return _NC_CACHE


_RUNNER = None
_ZEROS = None


def _get_runner():
    """Persistent jitted 8-core runner; compiled once per process."""
    global _RUNNER, _ZEROS
    if _RUNNER is None:
        import jax
        import jax.numpy as jnp
        from jax.sharding import Mesh, PartitionSpec, NamedSharding
        try:
            from jax import shard_map as _sm

            def _smap(f, mesh, in_specs, out_specs):
                return _sm(f, mesh=mesh, in_specs=in_specs,
                           out_specs=out_specs, check_vma=False)
        except ImportError:
            from jax.experimental.shard_map import shard_map as _sm

            def _smap(f, mesh, in_specs, out_specs):
                return _sm(f, mesh=mesh, in_specs=in_specs,
                           out_specs=out_specs, check_rep=False)
        from concourse.bass2jax import (
            _bass_exec_p,
            install_neuronx_cc_hook,
            partition_id_tensor,
        )

        install_neuronx_cc_hook()
        nc = _get_nc()
        pname = nc.partition_id_tensor.name if nc.partition_id_tensor else None

        def body(xc, gc, zc):
            ops = [xc, gc, zc]
            in_names = ["x", "gamma", "y"]
            if pname is not None:
                ops.append(partition_id_tensor())
                in_names.append(pname)
            (out,) = _bass_exec_p.bind(
                *ops,
                out_avals=(jax.core.ShapedArray((C, HW), jnp.float32),),
                in_names=tuple(in_names),
                out_names=("y",),
                lowering_input_output_aliases=(),
                sim_require_finite=True,
                sim_require_nnan=True,
                nc=nc,
            )
            return out

        devices = jax.devices()[:8]
        mesh = Mesh(np.asarray(devices), ("core",))
        runner = jax.jit(
            _smap(body, mesh, (PartitionSpec("core"),) * 3,
                  PartitionSpec("core"))
        )
        # zero "output" operand staged on device once; the kernel writes
        # every element of y, so the buffer contents are never observed.
        zeros = jax.device_put(
            np.zeros((8 * C, HW), np.float32),
            NamedSharding(mesh, PartitionSpec("core")),
        )
        jax.block_until_ready(zeros)
        _RUNNER, _ZEROS = runner, zeros
    return _RUNNER


def _kernel_fast(xf, gamma):
    f = _get_runner()
    gg = np.tile(gamma.reshape(1, 1), (8, 1))
    out = f(xf.reshape(8 * C, HW), gg, _ZEROS)
    return np.asarray(out).reshape(8, C, 64, 64)


def _kernel_spmd(xf, gamma):
    from concourse.bass_utils import run_bass_kernel_spmd

    nc = _get_nc()
    in_maps = [{"x": xf[b], "gamma": gamma} for b in range(8)]
    res = run_bass_kernel_spmd(nc, in_maps, core_ids=list(range(8)))
    out = np.stack([res.results[b]["y"] for b in range(8)], axis=0)
    return out.reshape(8, C, 64, 64).astype(np.float32, copy=False)


_FAST_OK = True


def kernel(x, gamma):
    global _FAST_OK
    x = np.ascontiguousarray(np.asarray(x, dtype=np.float32))
    assert x.shape == (8, C, 64, 64), x.shape
    xf = x.reshape(8, C, HW)
    gamma = np.ascontiguousarray(np.asarray(gamma, dtype=np.float32)).reshape(1)

    if _FAST_OK:
        try:
            return _kernel_fast(xf, gamma)
        except Exception:
            _FAST_OK = False
    return _kernel_spmd(xf, gamma)


# revision 3
# speedup vs baseline: 1.0974x; 1.0974x over previous
"""Trainium2 Bass kernel for nn_CAM (DANet channel-attention module).

Per batch element b (one per NeuronCore, 8 cores data-parallel over B=8):
    xf = x[b].reshape(C, H*W)                       # [512, 4096]
    E = xf @ xf.T                                   # [512, 512] (symmetric)
    att = softmax(max_j(E) - E, axis=-1)            # inverted softmax
    out = gamma * (att @ xf) + x[b]

Kernel math (identical in exact arithmetic to the reference):
    c[i]    = min_j E[i, j]         (= column min by symmetry)
    V[i, j] = exp(c[i] - E[i, j])   (exponent <= 0; att numerator^T)
    S[i]    = sum_j V[i, j]
    W       = V^T                   (= att numerator in [j, i] layout)
    out[i]  = gamma * (1/S[i]) * sum_j W[j, i] * xf[j, :] + x[b][i, :]

Device pipeline (f32r matmuls; bf16/fp8 E is numerically unacceptable —
the inverted softmax is a near-one-hot argmin over energies with tie
gaps ~1.0 vs E magnitude ~4096, so E needs ~f32 precision):
  - xf natural [c_part, n_free]  : [128, 4, 4096] f32 (residual source)
  - Xf = f32r-rounded copy       : transposes + mm2 rhs (the BIR
                                   verifier requires explicit rounding)
  - xf^T [n_part, c_free]        : rolling [128, 32, 512] f32r via PE
                                   transposes, fused with chunked loads
  - E [j_part, i_free]           : 4 PSUM banks, f32r matmuls over 32
                                   k-tiles; only the upper block-triangle
                                   (free dims >= 256 keeps f32r at
                                   1 cyc/row); lower blocks transposed in
  - V = exp(rowmin - E)          : scalar engine, per-partition bias;
                                   accum_out produces S in the same pass
                                   (no ones-matmuls, no DRAM roundtrip
                                   for a free-axis softmax shift)
  - W = V^T                      : 16 PE transposes, per i-block, so
                                   mm2's first weights are ready ~1.5us
                                   after the last k-tile
  - out                          : mm2 f32r + scalar_tensor_tensor
                                   (po * (gamma/S) + x); with gamma=0 the
                                   output is bit-exactly x

Host path: a persistent jitted shard_map runner (module-cached, built
once), device-resident zero output operands (no 64MB upload per call),
and no host-side stack/astype copies (input reshape and output reshape
are views). Falls back to concourse.bass_utils.run_bass_kernel_spmd on
any failure.

reps > 1 chains the computation through a DRAM scratch tile inside one
NEFF (used by test.py to measure steady-state per-iteration device
time).
"""

import numpy as np

import concourse.bass as bass
import concourse.mybir as mybir
import concourse.tile as tile
from concourse import bacc
from concourse.masks import make_identity

P = 128          # partitions
C = 512          # channels
HW = 4096        # spatial (64*64)
CB = C // P      # 4 channel blocks
KB = HW // P     # 32 spatial blocks
NW = 512         # matmul free-dim chunk
NCH = HW // NW   # 8 n-chunks

F32 = mybir.dt.float32
F32R = mybir.dt.float32r
EXP = mybir.ActivationFunctionType.Exp
ALU = mybir.AluOpType
AX = mybir.AxisListType


def build_nc(reps: int = 1):
    nc = bacc.Bacc("TRN2", target_bir_lowering=False)
    x = nc.dram_tensor("x", [C, HW], F32, kind="ExternalInput")
    g = nc.dram_tensor("gamma", [1], F32, kind="ExternalInput")
    y = nc.dram_tensor("y", [C, HW], F32, kind="ExternalOutput")

    with tile.TileContext(nc) as tc:
        with (
            tc.tile_pool(name="xin", bufs=1) as xin_pool,
            tc.tile_pool(name="xrp", bufs=1) as xr_pool,
            tc.tile_pool(name="xtr", bufs=6) as xtr_pool,
            tc.tile_pool(name="w", bufs=1) as w_pool,
            tc.tile_pool(name="small", bufs=1) as small,
            tc.tile_pool(name="outp", bufs=2) as outp,
            tc.tile_pool(name="dram", bufs=1, space="DRAM") as dramp,
            tc.tile_pool(name="pxt", bufs=2, space="PSUM") as pxt_pool,
            tc.tile_pool(name="acc", bufs=4, space="PSUM") as acc_pool,
        ):
            # constants (hoisted out of the rep loop)
            ident_f = small.tile([P, P], F32)
            make_identity(nc, ident_f)
            ident_r = small.tile([P, P], F32R)
            nc.scalar.copy(out=ident_r, in_=ident_f)
            gamma_bc = small.tile([P, 1], F32)
            nc.gpsimd.dma_start(out=gamma_bc, in_=g[:].partition_broadcast(P))

            xr = x.rearrange("(t p) n -> p t n", p=P)
            yr = y.rearrange("(t p) n -> p t n", p=P)

            if reps > 1:
                ybuf = dramp.tile([C, HW], F32, tag="ybuf")
                ybr = ybuf.rearrange("(t p) n -> p t n", p=P)

            for _rep in range(reps):
                in_r = xr if _rep == 0 else ybr
                out_r = yr if _rep == reps - 1 else ybr
                X = xin_pool.tile([P, CB, HW], F32, tag="x")
                Xf = xr_pool.tile([P, CB, HW], F32R, tag="xr")
                W = w_pool.tile([P, CB, C], F32R, tag="w")
                V = w_pool.tile([P, CB, C], F32R, tag="v")
                rowmin = small.tile([P, CB], F32, tag="rowmin")
                s_acc = small.tile([P, CB], F32, tag="sacc")
                invsg = small.tile([P, CB], F32, tag="invsg")

                # E accumulator banks (held across the fused load/T/mm1 loop)
                pe_tiles = [acc_pool.tile([P, C], F32, tag="acc", name=f"pe_{_jb}")
                            for _jb in range(CB)]
                # by symmetry only the upper block-triangle of E is computed
                # by matmuls; rhs column start per j-block (block (3,2) is
                # recomputed directly so every matmul keeps free dim >= 256)
                RS = (0, P, 2 * P, 2 * P)

                # ---- fused: load chunk -> round to f32r -> transposes -> mm1
                for ch in range(NCH):
                    subs = (
                        (slice(0, NW // 2), slice(NW // 2, NW))
                        if ch == 0 else (slice(0, NW),)
                    )
                    for sub in subs:
                        lo = ch * NW + sub.start
                        hi = ch * NW + sub.stop
                        nc.sync.dma_start(
                            out=X[:, :, lo:hi], in_=in_r[:, :, lo:hi]
                        )
                        nc.vector.tensor_copy(
                            out=Xf[:, :, lo:hi], in_=X[:, :, lo:hi]
                        )
                    for kk2 in range(NW // (2 * P)):
                        k0 = ch * (NW // P) + 2 * kk2
                        pxt = pxt_pool.tile([P, 2, C], F32R, tag="pxt")
                        for dk in range(2):
                            for t in range(CB):
                                nc.tensor.transpose(
                                    pxt[:, dk, t * P:(t + 1) * P],
                                    Xf[:, t, (k0 + dk) * P:(k0 + dk + 1) * P],
                                    ident_r,
                                )
                        xt2 = xtr_pool.tile([P, 2, C], F32R, tag="xtk")
                        if kk2 % 2 == 0:
                            nc.vector.tensor_copy(out=xt2, in_=pxt.bitcast(F32))
                        else:
                            nc.scalar.copy(out=xt2, in_=pxt.bitcast(F32))
                        for dk in range(2):
                            k = k0 + dk
                            for jb in range(CB):
                                nc.tensor.matmul(
                                    pe_tiles[jb][:, RS[jb]:],
                                    lhsT=xt2[:, dk, jb * P:(jb + 1) * P],
                                    rhs=xt2[:, dk, RS[jb]:],
                                    start=(k == 0),
                                    stop=(k == KB - 1),
                                )

                # ---- W chain: rowmin -> V = exp(rowmin - E) (+ S accum)
                # -> W = V^T blocks.  Block 0 needs no reconstruction, so its
                # rowmin/exp are emitted first and mm2's ib=0 weights are
                # ready shortly after the last k-tile (no DRAM roundtrip).
                def w_chain(ib):
                    nc.vector.tensor_reduce(
                        out=rowmin[:, ib:ib + 1], in_=pe_tiles[ib],
                        axis=AX.X, op=ALU.min,
                    )
                    nc.scalar.activation(
                        out=V[:, ib, :], in_=pe_tiles[ib], func=EXP,
                        bias=rowmin[:, ib:ib + 1], scale=-1.0,
                        accum_out=s_acc[:, ib:ib + 1],
                    )

                def w_transpose(ib):
                    # lands on pe_tiles[ib]'s PSUM bank (acc pool rotation);
                    # every reader of that bank is emitted above.
                    wtp = acc_pool.tile([P, CB, P], F32R, tag="acc")
                    for jb in range(CB):
                        nc.tensor.transpose(
                            wtp[:, jb, :], V[:, ib, jb * P:(jb + 1) * P], ident_r
                        )
                    isl = slice(ib * P, (ib + 1) * P)
                    if ib % 2 == 0:
                        nc.vector.tensor_copy(
                            out=W[:, :, isl], in_=wtp.bitcast(F32)
                        )
                    else:
                        nc.scalar.copy(out=W[:, :, isl], in_=wtp.bitcast(F32))

                w_chain(0)
                # reconstruct the lower block-triangle: E[i,j] = E[j,i]^T
                blk = small.tile([P, 5, P], F32, tag="blk")
                for n5, (bi, bj) in enumerate(((1, 0), (2, 0), (2, 1), (3, 0), (3, 1))):
                    nc.vector.tensor_copy(
                        out=blk[:, n5, :], in_=pe_tiles[bj][:, bi * P:(bi + 1) * P]
                    )
                    nc.tensor.transpose(
                        pe_tiles[bi][:, bj * P:(bj + 1) * P], blk[:, n5, :], ident_f
                    )
                for ib in range(1, CB):
                    w_chain(ib)

                # ---- phase 2: out = gamma * (1/S) * (W^T @ xf) + x
                # w_transpose(ib) and mm2(ib) are interleaved in the PE
                # instruction stream: mm2 of block ib only needs block ib's
                # weights, and its ~6.8us of matmuls hide the remaining
                # blocks' rowmin/exp chains (the PE queue is in-order, so
                # emitting all 16 W-transposes up front would stall the PE
                # on block 3's scalar chain before mm2 could start).
                for ib in range(CB):
                    isl = slice(ib * P, (ib + 1) * P)
                    w_transpose(ib)
                    nc.vector.reciprocal(
                        out=invsg[:, ib:ib + 1], in_=s_acc[:, ib:ib + 1]
                    )
                    nc.vector.tensor_tensor(
                        out=invsg[:, ib:ib + 1], in0=invsg[:, ib:ib + 1],
                        in1=gamma_bc, op=ALU.mult,
                    )
                    out_sb = outp.tile([P, HW], F32, tag="osb")
                    for chn in range(NCH):
                        nsl = slice(chn * NW, (chn + 1) * NW)
                        po_t = acc_pool.tile([P, NW], F32, tag="acc")
                        for jb in range(CB):
                            nc.tensor.matmul(
                                po_t,
                                lhsT=W[:, jb, isl],
                                rhs=Xf[:, jb, nsl],
                                start=(jb == 0),
                                stop=(jb == CB - 1),
                            )
                        nc.vector.scalar_tensor_tensor(
                            out=out_sb[:, nsl],
                            in0=po_t,
                            scalar=invsg[:, ib:ib + 1],
                            in1=X[:, ib, nsl],
                            op0=ALU.mult,
                            op1=ALU.add,
                        )
                        nc.scalar.dma_start(
                            out=out_r[:, ib, nsl], in_=out_sb[:, nsl]
                        )

    nc.compile()
    return nc


_NC_CACHE = None


def _get_nc():
    global _NC_CACHE
    if _NC_CACHE is None:
        _NC_CACHE = build_nc()
    return _NC_CACHE


_RUNNER = None
_ZEROS = None


def _get_runner():
    """Persistent jitted 8-core runner; compiled once per process."""
    global _RUNNER, _ZEROS
    if _RUNNER is None:
        import jax
        import jax.numpy as jnp
        from jax.sharding import Mesh, PartitionSpec, NamedSharding
        try:
            from jax import shard_map as _sm

            def _smap(f, mesh, in_specs, out_specs):
                return _sm(f, mesh=mesh, in_specs=in_specs,
                           out_specs=out_specs, check_vma=False)
        except ImportError:
            from jax.experimental.shard_map import shard_map as _sm

            def _smap(f, mesh, in_specs, out_specs):
                return _sm(f, mesh=mesh, in_specs=in_specs,
                           out_specs=out_specs, check_rep=False)
        from concourse.bass2jax import (
            _bass_exec_p,
            install_neuronx_cc_hook,
            partition_id_tensor,
        )

        install_neuronx_cc_hook()
        nc = _get_nc()
        pname = nc.partition_id_tensor.name if nc.partition_id_tensor else None

        def body(xc, gc, zc):
            ops = [xc, gc, zc]
            in_names = ["x", "gamma", "y"]
            if pname is not None:
                ops.append(partition_id_tensor())
                in_names.append(pname)
            (out,) = _bass_exec_p.bind(
                *ops,
                out_avals=(jax.core.ShapedArray((C, HW), jnp.float32),),
                in_names=tuple(in_names),
                out_names=("y",),
                lowering_input_output_aliases=(),
                sim_require_finite=True,
                sim_require_nnan=True,
                nc=nc,
            )
            return out

        devices = jax.devices()[:8]
        mesh = Mesh(np.asarray(devices), ("core",))
        runner = jax.jit(
            _smap(body, mesh, (PartitionSpec("core"),) * 3,
                  PartitionSpec("core"))
        )
        # zero "output" operand staged on device once; the kernel writes
        # every element of y, so the buffer contents are never observed.
        zeros = jax.device_put(
            np.zeros((8 * C, HW), np.float32),
            NamedSharding(mesh, PartitionSpec("core")),
        )
        jax.block_until_ready(zeros)
        _RUNNER, _ZEROS = runner, zeros
    return _RUNNER


def _kernel_fast(xf, gamma):
    f = _get_runner()
    gg = np.tile(gamma.reshape(1, 1), (8, 1))
    out = f(xf.reshape(8 * C, HW), gg, _ZEROS)
    return np.asarray(out).reshape(8, C, 64, 64)


def _kernel_spmd(xf, gamma):
    from concourse.bass_utils import run_bass_kernel_spmd

    nc = _get_nc()
    in_maps = [{"x": xf[b], "gamma": gamma} for b in range(8)]
    res = run_bass_kernel_spmd(nc, in_maps, core_ids=list(range(8)))
    out = np.stack([res.results[b]["y"] for b in range(8)], axis=0)
    return out.reshape(8, C, 64, 64).astype(np.float32, copy=False)


_FAST_OK = True


def kernel(x, gamma):
    global _FAST_OK
    x = np.ascontiguousarray(np.asarray(x, dtype=np.float32))
    assert x.shape == (8, C, 64, 64), x.shape
    xf = x.reshape(8, C, HW)
    gamma = np.ascontiguousarray(np.asarray(gamma, dtype=np.float32)).reshape(1)

    if _FAST_OK:
        try:
            return _kernel_fast(xf, gamma)
        except Exception:
            _FAST_OK = False
    return _kernel_spmd(xf, gamma)
